# revision 26
# baseline (speedup 1.0000x reference)
"""Trainium2 Bass kernel for nn_BaseContrastHead (MoCo-style contrastive loss).

Strategy (8 NeuronCores, data-parallel per sharding hint):
  - Shard the 8192 sample dim of feats_weak/feats_strong (+labels) contiguously
    across 8 cores (1024 rows each). Queue + projection weights replicated.
  - Label-matching index computed ON DEVICE via one-hot matmuls:
      per-128-block class histograms -> AllGather core totals -> global
      per-class prefix counts -> occurrence ranks occ/rank (E-matmul + tril) ->
      class/rank lookup tables G_hi/G_lo (one-hot matmuls, j split into
      hi/lo 6-bit halves so all matmul values stay bf16-exact) -> AllGather G ->
      per-weak-row fused gather matmul [rank_pre|cnt|H] -> idx.
  - kT = Wk.T @ featsT_s accumulated chunk-by-chunk as DMA lands, PE-transposed
    to sample-major, AllGather k (MoCo-style). qT likewise (kept emb-major).
  - ksel = indirect-DMA row gather of k_full by idx; l_pos via PE transpose +
    ones-matmul; l_neg = qT.T @ queue.reshape(128,1024); CE with fixed -36
    shift (no max pass), batched [128,8] softmax tail (one Exp->Ln table
    transition); per-core (sum_ce, valid_count) partials -> host combine.
"""

import numpy as np
import ml_dtypes

# ---- problem dims (hardcoded per contract) ----
N = 8192          # samples (weak == strong count)
FD = 1024         # encoder input feature dim
EMB = 128         # embedding size
QN = 1024         # queue rows
C = 80            # num classes
NCORE = 8
NSH = N // NCORE  # 1024 rows per core
P = 128           # partition dim / block size
B = NSH // P      # 8 blocks per core
R = 160           # rank-table width (max per-class count is ~135 w.h.p.)
BF = ml_dtypes.bfloat16
SHIFT = -36.0     # softmax exp shift (max |logit| ~ 68)

_CACHE = {}


def _build_program(stop_after="full"):
    import concourse.bass as bass
    import concourse.tile as tile
    from concourse import bacc, mybir
    from concourse.bass import IndirectOffsetOnAxis

    f32 = mybir.dt.float32
    bf16 = mybir.dt.bfloat16
    i32 = mybir.dt.int32
    EQ = mybir.AluOpType.is_equal
    GE = mybir.AluOpType.is_ge
    GT = mybir.AluOpType.is_gt
    MUL = mybir.AluOpType.mult
    ADD = mybir.AluOpType.add
    SUB = mybir.AluOpType.subtract
    AX = mybir.AxisListType.X
    ACT = mybir.ActivationFunctionType

    nc = bacc.Bacc("TRN2", target_bir_lowering=False, debug=False,
                   num_devices=NCORE)

    # ---------------- I/O ----------------
    def inp(name, shape, dt):
        return nc.dram_tensor(name, shape, dt, kind="ExternalInput").ap()

    ftw = inp("ftw", [FD, NSH], bf16)        # feats_weak shard, transposed
    fts = inp("fts", [FD, NSH], bf16)        # feats_strong shard, transposed
    slab = inp("slab", [NSH, 1], f32)        # strong labels (scalar use)
    slabb = inp("slabb", [NSH, 1], bf16)     # strong labels (broadcast use)
    wlabb = inp("wlabb", [NSH, 1], bf16)     # weak labels (broadcast use)
    wq = inp("wq", [FD, EMB], bf16)
    wk = inp("wk", [FD, EMB], bf16)
    q2 = inp("q2", [EMB, QN], bf16)          # queue.reshape(128, 1024)
    cblobf = inp("cblobf", [P, 21], f32)     # packed f32 constants
    cblobb = inp("cblobb", [P, 1425], bf16)  # packed bf16 constants
    out_d = nc.dram_tensor("out", [1, 2], f32, kind="ExternalOutput").ap()

    # ---------------- internal DRAM (collectives) ----------------
    hist_in = nc.dram_tensor("hist_in", [C, 2], f32).ap()
    hist_ag = nc.dram_tensor("hist_ag", [NCORE * C, 2], f32,
                             addr_space="Shared").ap()
    g_in = nc.dram_tensor("g_in", [C, 2 * R], bf16).ap()
    g_ag = nc.dram_tensor("g_ag", [NCORE * C, 2 * R], bf16,
                          addr_space="Shared").ap()
    kb = nc.dram_tensor("kb", [NSH, EMB], bf16).ap()
    warm_in = nc.dram_tensor("warm_in", [8, 1], f32).ap()
    warm_ag = nc.dram_tensor("warm_ag", [NCORE * 8, 1], f32,
                             addr_space="Shared").ap()
    kfull = nc.dram_tensor("kfull", [N, EMB], bf16, addr_space="Shared").ap()

    RG = [list(range(NCORE))]

    from contextlib import ExitStack

    with tile.TileContext(nc) as tc, ExitStack() as ctx:
        cp = ctx.enter_context(tc.tile_pool(name="const", bufs=1))
        pp = ctx.enter_context(tc.tile_pool(name="persist", bufs=1))
        lp = ctx.enter_context(tc.tile_pool(name="loop", bufs=3))
        sp = ctx.enter_context(tc.tile_pool(name="small", bufs=4))
        psP = ctx.enter_context(tc.tile_pool(name="psP", bufs=1, space="PSUM"))
        psA = ctx.enter_context(tc.tile_pool(name="psA", bufs=2, space="PSUM"))
        psS = ctx.enter_context(tc.tile_pool(name="psS", bufs=2, space="PSUM"))

        dma = nc.sync.dma_start      # big streaming ring
        dma2 = nc.scalar.dma_start   # side-channel ring
        v = nc.vector

        # whole-shard label broadcasts + natural strong labels: 3 DMAs
        sb_all = cp.tile([C, NSH], bf16, tag="sball")
        dma(sb_all[:], slabb.rearrange("a b -> b a").to_broadcast((C, NSH)))
        wb_all = cp.tile([C, NSH], bf16, tag="wball")
        dma(wb_all[:], wlabb.rearrange("a b -> b a").to_broadcast((C, NSH)))
        sn_all = cp.tile([P, B], f32, tag="snall")
        dma(sn_all[:], slab.rearrange("(b p) one -> p (b one)", p=P))

        # -------- packed consts: two DMAs total (side ring, first) --------
        cf = cp.tile([P, 21], f32, tag="cf")
        dma2(cf[:], cblobf)
        cb = cp.tile([P, 1425], bf16, tag="cb")
        dma2(cb[:], cblobb)
        i80c = cf[0:C, 0:1]
        cmask_sb = cf[0:C, 1:9]
        jhi_sb = cf[:, 9:17]
        jlo_sb = cf[:, 17:18]
        onesf_sb = cf[:, 18:19]
        negs_sb = cf[:, 19:20]
        zero_sb = cf[:, 20:21]
        tril_sb = cb[:, 0:P]
        ident_sb = cb[:, P:2 * P]
        onesc_sb = cb[:, 2 * P:2 * P + 1]
        onesr_sb = cb[0:1, 257:769]
        bq_sb = cb[0:1, 769:897]
        bk_sb = cb[0:1, 897:1025]
        i320b = cb[:, 1025:1345]
        i80rb = cb[:, 1345:1425]
        # ---------------- persistent tiles ----------------
        skeepT = pp.tile([C, B * P], bf16, tag="skeepT")    # S1hT per block
        wkeepT = pp.tile([C, B * P], bf16, tag="wkeepT")    # W1hT per block
        s1hkeep = pp.tile([P, B * C], bf16, tag="s1hkeep")  # S1h natural
        occin = pp.tile([P, B], f32, tag="occin")
        rankin = pp.tile([P, B], f32, tag="rankin")
        hist_s = pp.tile([C, B], f32, tag="hist_s")
        hist_w = pp.tile([C, B], f32, tag="hist_w")
        gacc = pp.tile([C, 2 * R], bf16, tag="gacc")
        gsum = pp.tile([C, 2 * R], bf16, tag="gsum")
        qT_sb = pp.tile([EMB, NSH], bf16, tag="qT")
        kT_sb = pp.tile([EMB, NSH], bf16, tag="kT")
        k_sb = pp.tile([P, B * EMB], bf16, tag="ksb")
        idx_all = pp.tile([P, B], i32, tag="idx")
        vmask = pp.tile([P, B], f32, tag="vmask")
        rankpre = pp.tile([P, B], f32, tag="rankpre")
        cnt_all = pp.tile([P, B], f32, tag="cntall")
        ih_all = pp.tile([P, B], f32, tag="ihall")
        il_all = pp.tile([P, B], f32, tag="ilall")
        lpos_all = pp.tile([P, B], f32, tag="lposall")
        h_keep = pp.tile([P, B * 2 * R], bf16, tag="hkeep")
        sneg_all = pp.tile([P, 2 * B], f32, tag="snegall")
        cesum = pp.tile([P, 2], f32, tag="cesum")

        class _EarlyStop(Exception):
            pass

        _LVL = {"lab1": 1, "hist": 2, "proj": 2.5, "gtable": 3, "weak": 4,
                "full": 5}[stop_after]

        def ckpt(n):
            if _LVL <= n:
                fin0 = sp.tile([1, 2], f32, tag="finsb")
                nc.gpsimd.memset(fin0[:], 0.0)
                dma2(out_d, fin0[:])
                raise _EarlyStop

        try:
            # warmup collective: absorbs ncfw first-call latency in parallel
            wz = sp.tile([8, 1], f32, tag="warmz")
            nc.gpsimd.memset(wz[:], 0.0)
            dma2(warm_in, wz[:])
            nc.gpsimd.collective_compute(
                "AllGather", mybir.AluOpType.bypass, replica_groups=RG,
                ins=[warm_in], outs=[warm_ag])
            # ====== critical label path: one-shot one-hots -> hist AG ======
            v.tensor_scalar(out=skeepT[:], in0=sb_all[:], scalar1=i80c,
                            scalar2=None, op0=EQ)
            v.tensor_scalar(out=wkeepT[:], in0=wb_all[:], scalar1=i80c,
                            scalar2=None, op0=EQ)
            coretot = sp.tile([C, 2], f32, tag="coretot")
            v.reduce_sum(out=coretot[:, 0:1], in_=skeepT[:], axis=AX)
            v.reduce_sum(out=coretot[:, 1:2], in_=wkeepT[:], axis=AX)
            dma2(hist_in, coretot[:])
            ckpt(1)
            nc.gpsimd.collective_compute(
                "AllGather", mybir.AluOpType.bypass, replica_groups=RG,
                ins=[hist_in], outs=[hist_ag])

            # ====== rest of label phase 1 (off the AG critical path) ======
            for b in range(B):
                bs = slice(b * P, (b + 1) * P)
                v.tensor_scalar(out=s1hkeep[:, b * C: (b + 1) * C], in0=i80rb,
                                scalar1=sn_all[:, b: b + 1], scalar2=None,
                                op0=EQ)
                v.reduce_sum(out=hist_s[:, b: b + 1], in_=skeepT[:, bs], axis=AX)
                eps = psS.tile([P, P], f32, tag="mm")
                nc.tensor.matmul(eps[:], lhsT=skeepT[:, bs], rhs=skeepT[:, bs],
                                 start=True, stop=True)
                scr = lp.tile([P, P], bf16, tag="escr")
                v.tensor_tensor(out=scr[:], in0=eps[:], in1=tril_sb, op=MUL)
                v.reduce_sum(out=occin[:, b: b + 1], in_=scr[:], axis=AX)

                v.reduce_sum(out=hist_w[:, b: b + 1], in_=wkeepT[:, bs], axis=AX)
                epw = psS.tile([P, P], f32, tag="mm")
                nc.tensor.matmul(epw[:], lhsT=wkeepT[:, bs], rhs=wkeepT[:, bs],
                                 start=True, stop=True)
                scw = lp.tile([P, P], bf16, tag="escr")
                v.tensor_tensor(out=scw[:], in0=epw[:], in1=tril_sb, op=MUL)
                v.reduce_sum(out=rankin[:, b: b + 1], in_=scw[:], axis=AX)

            # ====== projections: chunked K accumulation as DMA lands ======
            wq_sb = cp.tile([P, 8 * EMB], bf16, tag="wq")
            dma(wq_sb[:], wq.rearrange("(k p) e -> p k e", p=P))
            wk_sb = cp.tile([P, 8 * EMB], bf16, tag="wk")
            dma(wk_sb[:], wk.rearrange("(k p) e -> p k e", p=P))
            q2_sb = cp.tile([EMB, QN], bf16, tag="q2")
            dma(q2_sb[:], q2)
            fts_sb = pp.tile([P, 8 * NSH], bf16, tag="fts")
            ftw_sb = pp.tile([P, 8 * NSH], bf16, tag="ftw")
            fts_r = fts.rearrange("(k p) s -> p k s", p=P)
            ftw_r = ftw.rearrange("(k p) s -> p k s", p=P)

            kacc = psP.tile([EMB, NSH], f32, tag="kacc")
            qacc = psP.tile([EMB, NSH], f32, tag="qacc")
            for kk in range(8):
                if kk % 2 == 0:
                    cs = slice(kk * NSH, (kk + 2) * NSH)
                    dma(fts_sb[:, cs], fts_r[:, kk: kk + 2, :])
                    dma(ftw_sb[:, cs], ftw_r[:, kk: kk + 2, :])
                for h in range(2):
                    hs = slice(h * 512, (h + 1) * 512)
                    nc.tensor.matmul(
                        kacc[:, hs], lhsT=wk_sb[:, kk * EMB: (kk + 1) * EMB],
                        rhs=fts_sb[:, kk * NSH + h * 512:
                                   kk * NSH + (h + 1) * 512],
                        start=(kk == 0), stop=False, skip_group_check=True)
                for h in range(2):
                    hs = slice(h * 512, (h + 1) * 512)
                    nc.tensor.matmul(
                        qacc[:, hs], lhsT=wq_sb[:, kk * EMB: (kk + 1) * EMB],
                        rhs=ftw_sb[:, kk * NSH + h * 512:
                                   kk * NSH + (h + 1) * 512],
                        start=(kk == 0), stop=False, skip_group_check=True)
            for h in range(2):
                hs = slice(h * 512, (h + 1) * 512)
                nc.tensor.matmul(kacc[:, hs], lhsT=bk_sb,
                                 rhs=onesr_sb, start=False, stop=True,
                                 skip_group_check=True)
                nc.tensor.matmul(qacc[:, hs], lhsT=bq_sb,
                                 rhs=onesr_sb, start=False, stop=True,
                                 skip_group_check=True)
                nc.scalar.copy(out=kT_sb[:, hs], in_=kacc[:, hs])
                nc.scalar.copy(out=qT_sb[:, hs], in_=qacc[:, hs])

            # kT -> k natural (PE transpose), write shard, AllGather
            for b in range(B):
                kn = psS.tile([P, P], bf16, tag="mm")
                nc.tensor.transpose(out=kn[:], in_=kT_sb[:, b * P: (b + 1) * P],
                                    identity=ident_sb)
                nc.scalar.copy(out=k_sb[:, b * EMB: (b + 1) * EMB], in_=kn[:])
            dma2(kb.rearrange("(b p) e -> p b e", p=P), k_sb[:])
            if _LVL <= 2.5:
                ckpt(2.4)
            nc.gpsimd.collective_compute(
                "AllGather", mybir.AluOpType.bypass, replica_groups=RG,
                ins=[kb], outs=[kfull])

            # ====== l_neg + exp (only needs qT + queue; runs early on ACT) ==
            for b in range(B):
                qslice = qT_sb[:, b * P: (b + 1) * P]
                for h in range(2):
                    lneg = psA.tile([P, 512], f32, tag="lneg")
                    nc.tensor.matmul(lneg[:], lhsT=qslice,
                                     rhs=q2_sb[:, h * 512: (h + 1) * 512],
                                     start=True, stop=True,
                                     skip_group_check=True)
                    ebuf = lp.tile([P, 512], f32, tag="ebuf")
                    nc.scalar.activation(
                        out=ebuf[:], in_=lneg[:], func=ACT.Exp, bias=negs_sb,
                        accum_out=sneg_all[:, h * B + b: h * B + b + 1])

            ckpt(2)
            # ====== cross-core prefix bases ======
            def excl_cumsum(src, tagp):
                cur = src
                for s in (1, 2, 4):
                    nxt = sp.tile([C, B], f32, tag=tagp)
                    v.tensor_tensor(out=nxt[:, s:], in0=cur[:, s:],
                                    in1=cur[:, :B - s], op=ADD)
                    v.tensor_copy(out=nxt[:, :s], in_=cur[:, :s])
                    cur = nxt
                ex = sp.tile([C, B], f32, tag=tagp)
                v.tensor_tensor(out=ex[:], in0=cur[:], in1=src[:], op=SUB)
                return cur, ex  # inclusive, exclusive

            hist_r3 = hist_ag.rearrange("(m c) s -> c m s", m=NCORE)
            prevecs = {}
            for side, hown, scol in (("s", hist_s, 0), ("w", hist_w, 1)):
                ct = sp.tile([C, NCORE], f32, tag="ct" + side)
                dma2(ct[:], hist_r3[:, :, scol: scol + 1])
                ct_inc, ct_ex = excl_cumsum(ct, "ctp" + side)
                scrb = sp.tile([C, NCORE], f32, tag="scrb" + side)
                base = sp.tile([C, 1], f32, tag="base" + side)
                v.tensor_tensor(out=scrb[:], in0=ct_ex[:], in1=cmask_sb[:],
                                op=MUL)
                v.reduce_sum(out=base[:], in_=scrb[:], axis=AX)
                if side == "s":
                    tots_bf = sp.tile([C, 1], bf16, tag="tots")
                    v.tensor_copy(out=tots_bf[:],
                                  in_=ct_inc[:, NCORE - 1: NCORE])
                _, own_ex = excl_cumsum(hown, "ownp" + side)
                pv = pp.tile([C, B], bf16, tag="prevec" + side)
                v.tensor_scalar(out=pv[:], in0=own_ex[:], scalar1=base[:],
                                scalar2=None, op0=ADD)
                prevecs[side] = pv
            prevec_s, prevec_w = prevecs["s"], prevecs["w"]

            # ====== strong phase 2: occ -> G tables (PSUM-accumulated) ======
            gps = psP.tile([C, 2 * R], f32, tag="qacc")  # reuse qacc slot
            for b in range(B):
                bs = slice(b * P, (b + 1) * P)
                ops_ = psS.tile([P, 1], f32, tag="mm")
                nc.tensor.matmul(ops_[:], lhsT=skeepT[:, bs],
                                 rhs=prevec_s[:, b: b + 1], start=True,
                                 stop=True, skip_group_check=True)
                occf = lp.tile([P, 1], f32, tag="occf")
                v.tensor_tensor(out=occf[:], in0=occin[:, b: b + 1],
                                in1=ops_[:], op=ADD)
                o1hj = lp.tile([P, 2 * R], bf16, tag="o1hj")
                v.tensor_scalar(out=o1hj[:, :R], in0=i320b[:, :R],
                                scalar1=occf[:],
                                scalar2=jhi_sb[:, b: b + 1], op0=EQ, op1=MUL)
                v.tensor_scalar(out=o1hj[:, R:], in0=i320b[:, :R],
                                scalar1=occf[:],
                                scalar2=jlo_sb, op0=EQ, op1=MUL)
                nc.tensor.matmul(gps[:], lhsT=s1hkeep[:, b * C: (b + 1) * C],
                                 rhs=o1hj[:], start=(b == 0),
                                 stop=(b == B - 1), skip_group_check=True)
            v.tensor_copy(out=gacc[:], in_=gps[:])
            dma2(g_in, gacc[:])
            ckpt(2.6)
            nc.gpsimd.collective_compute(
                "AllGather", mybir.AluOpType.bypass, replica_groups=RG,
                ins=[g_in], outs=[g_ag])
            gall = pp.tile([C, NCORE * 2 * R], bf16, tag="gall")
            dma2(gall[:], g_ag.rearrange("(m c) r -> c m r", m=NCORE))
            v.tensor_copy(out=gsum[:], in_=gall[:, 0: 2 * R])
            for m in range(1, NCORE):
                v.tensor_tensor(out=gsum[:], in0=gsum[:],
                                in1=gall[:, m * 2 * R: (m + 1) * 2 * R], op=ADD)

            ckpt(3)
            # ====== weak phase 2: fused [rank_pre|cnt|H] matmul -> idx ======
            rhs_w = pp.tile([C, 2 + 2 * R], bf16, tag="rhsw")
            v.tensor_copy(out=rhs_w[:, 1:2], in_=tots_bf[:])
            v.tensor_copy(out=rhs_w[:, 2:], in_=gsum[:])
            for b in range(B):
                bs = slice(b * P, (b + 1) * P)
                v.tensor_copy(out=rhs_w[:, 0:1], in_=prevec_w[:, b: b + 1])
                hps = psS.tile([P, 2 + 2 * R], f32, tag="mm")
                nc.tensor.matmul(hps[:], lhsT=wkeepT[:, bs], rhs=rhs_w[:],
                                 start=True, stop=True, skip_group_check=True)
                v.tensor_copy(out=rankpre[:, b: b + 1], in_=hps[:, 0:1])
                v.tensor_copy(out=cnt_all[:, b: b + 1], in_=hps[:, 1:2])
                nc.scalar.copy(out=h_keep[:, b * 2 * R: (b + 1) * 2 * R],
                               in_=hps[:, 2:])
            # batched rank/cnt/sel
            v.tensor_scalar(out=vmask[:], in0=cnt_all[:], scalar1=0.0,
                            scalar2=None, op0=GT)
            rank_all = sp.tile([P, B], f32, tag="rankall")
            v.tensor_tensor(out=rank_all[:], in0=rankin[:], in1=rankpre[:],
                            op=ADD)
            sel = rank_all
            for _ in range(2):
                ge = sp.tile([P, B], f32, tag="selge")
                v.tensor_tensor(out=ge[:], in0=sel[:], in1=cnt_all[:], op=GE)
                sub = sp.tile([P, B], f32, tag="selsub")
                v.tensor_tensor(out=sub[:], in0=ge[:], in1=cnt_all[:], op=MUL)
                nsel = sp.tile([P, B], f32, tag="selnew")
                v.tensor_tensor(out=nsel[:], in0=sel[:], in1=sub[:], op=SUB)
                sel = nsel
            for b in range(B):
                o1hw = lp.tile([P, 2 * R], bf16, tag="o1hw")
                v.tensor_scalar(out=o1hw[:], in0=i320b,
                                scalar1=sel[:, b: b + 1], scalar2=None, op0=EQ)
                scr2 = lp.tile([P, 2 * R], bf16, tag="hscr")
                v.tensor_tensor(out=scr2[:],
                                in0=h_keep[:, b * 2 * R: (b + 1) * 2 * R],
                                in1=o1hw[:], op=MUL)
                v.reduce_sum(out=ih_all[:, b: b + 1], in_=scr2[:, :R], axis=AX)
                v.reduce_sum(out=il_all[:, b: b + 1], in_=scr2[:, R:], axis=AX)
            idf = sp.tile([P, B], f32, tag="idf")
            v.tensor_scalar(out=idf[:], in0=ih_all[:], scalar1=64.0,
                            scalar2=None, op0=MUL)
            idf2 = sp.tile([P, B], f32, tag="idf2")
            v.tensor_tensor(out=idf2[:], in0=idf[:], in1=il_all[:], op=ADD)
            v.tensor_copy(out=idx_all[:], in_=idf2[:])

            ckpt(4)
            # ====== sampling: one batched gather, then l_pos ======
            ksel_all = pp.tile([P, B * EMB], bf16, tag="kselall")
            nc.gpsimd.indirect_dma_start(
                out=ksel_all[:], out_offset=None, in_=kfull,
                in_offset=IndirectOffsetOnAxis(ap=idx_all[:, 0:B], axis=0))
            for b in range(B):
                kt = psS.tile([P, P], bf16, tag="mm")
                nc.tensor.transpose(out=kt[:],
                                    in_=ksel_all[:, b * EMB: (b + 1) * EMB],
                                    identity=ident_sb)
                prod = lp.tile([P, P], bf16, tag="prod")
                v.tensor_tensor(out=prod[:], in0=qT_sb[:, b * P: (b + 1) * P],
                                in1=kt[:], op=MUL)
                lpos = psS.tile([P, 1], f32, tag="mm")
                nc.tensor.matmul(lpos[:], lhsT=prod[:], rhs=onesc_sb,
                                 start=True, stop=True, skip_group_check=True)
                v.tensor_copy(out=lpos_all[:, b: b + 1], in_=lpos[:])

            # ====== batched softmax tail ======
            ep_all = sp.tile([P, B], f32, tag="epall")
            nc.scalar.activation(out=ep_all[:], in_=lpos_all[:], func=ACT.Exp,
                                 bias=negs_sb)
            z1 = sp.tile([P, B], f32, tag="z1")
            v.tensor_tensor(out=z1[:], in0=sneg_all[:, 0:B],
                            in1=sneg_all[:, B:], op=ADD)
            z = sp.tile([P, B], f32, tag="z")
            v.tensor_tensor(out=z[:], in0=z1[:], in1=ep_all[:], op=ADD)
            lz = sp.tile([P, B], f32, tag="lz")
            nc.scalar.activation(out=lz[:], in_=z[:], func=ACT.Ln,
                                 bias=zero_sb)
            ce0 = sp.tile([P, B], f32, tag="ce0")
            v.tensor_tensor(out=ce0[:], in0=lz[:], in1=lpos_all[:], op=SUB)
            ce = sp.tile([P, B], f32, tag="ce")
            v.tensor_scalar(out=ce[:], in0=ce0[:], scalar1=-SHIFT,
                            scalar2=None, op0=ADD)
            cem = sp.tile([P, B], f32, tag="cem")
            v.tensor_tensor(out=cem[:], in0=ce[:], in1=vmask[:], op=MUL)
            v.reduce_sum(out=cesum[:, 0:1], in_=cem[:], axis=AX)
            v.reduce_sum(out=cesum[:, 1:2], in_=vmask[:], axis=AX)

            # ====== final partition reduce -> [1, 2] ======
            ops2 = psS.tile([1, 2], f32, tag="mm")
            nc.tensor.matmul(ops2[:], lhsT=onesf_sb, rhs=cesum[:],
                             start=True, stop=True, skip_group_check=True)
            fin_sb = sp.tile([1, 2], f32, tag="finsb")
            v.tensor_copy(out=fin_sb[:], in_=ops2[:])
            dma2(out_d, fin_sb[:])
        except _EarlyStop:
            pass

    nc.compile()
    return nc


def _host_inputs(inputs):
    """Shard + cast host-side. Returns list of per-core input dicts."""
    fw = np.asarray(inputs["feats_weak"], np.float32)
    fs = np.asarray(inputs["feats_strong"], np.float32)
    wl = np.asarray(inputs["weak_labels"]).astype(np.float32)
    sl = np.asarray(inputs["strong_labels"]).astype(np.float32)
    Wq = np.asarray(inputs["Wq"], np.float32).astype(BF)
    Wk = np.asarray(inputs["Wk"], np.float32).astype(BF)
    bq = np.asarray(inputs["bq"], np.float32).astype(BF)
    bk = np.asarray(inputs["bk"], np.float32).astype(BF)
    q2 = np.ascontiguousarray(
        np.asarray(inputs["queue"], np.float32).reshape(EMB, QN)).astype(BF)

    # bf16 constant blob [128, 1425]
    cbb = np.zeros((P, 1425), np.float32)
    cbb[:, 0:P] = np.tril(np.ones((P, P)), -1)
    cbb[:, P:2 * P] = np.eye(P)
    cbb[:, 2 * P] = 1.0                       # ones column
    cbb[0, 257:769] = 1.0                     # ones row
    cbb[0, 769:897] = 0.0                     # bq placeholder (set below)
    cbb[0, 897:1025] = 0.0
    cbb[:, 1025:1345] = np.concatenate([np.arange(R), np.arange(R)])[None, :]
    cbb[:, 1345:1425] = np.arange(C)[None, :]
    cbb = cbb.astype(BF)
    cbb[0, 769:897] = bq
    cbb[0, 897:1025] = bk

    in_maps = []
    for m in range(NCORE):
        rows = slice(m * NSH, (m + 1) * NSH)
        ftw = np.ascontiguousarray(fw[rows].T).astype(BF)
        fts = np.ascontiguousarray(fs[rows].T).astype(BF)
        # f32 constant blob [128, 21] (per-core: cmask + jhi)
        cbf = np.zeros((P, 21), np.float32)
        cbf[0:C, 0] = np.arange(C)                     # i80c
        cbf[0:C, 1 + m] = 1.0                          # cmask one-hot
        jg = (np.arange(m * NSH, (m + 1) * NSH) // 64).astype(np.float32)
        cbf[:, 9:17] = jg.reshape(B, P).T              # jhi per (p, b)
        cbf[:, 17] = np.arange(P) % 64                 # jlo
        cbf[:, 18] = 1.0                               # onesf
        cbf[:, 19] = SHIFT
        cbf[:, 20] = 0.0
        in_maps.append({
            "ftw": ftw, "fts": fts,
            "slab": sl[rows].astype(np.float32).reshape(NSH, 1),
            "slabb": sl[rows].astype(BF).reshape(NSH, 1),
            "wlabb": wl[rows].astype(BF).reshape(NSH, 1),
            "wq": Wq, "wk": Wk, "q2": q2,
            "cblobf": cbf, "cblobb": cbb,
        })
    return in_maps


def kernel(**inputs):
    from concourse.bass_utils import run_bass_kernel_spmd

    if "nc" not in _CACHE:
        _CACHE["nc"] = _build_program()
    nc = _CACHE["nc"]
    in_maps = _host_inputs(inputs)
    res = run_bass_kernel_spmd(nc, in_maps, core_ids=list(range(NCORE)))
    _CACHE["last_results"] = res
    parts = np.stack([r["out"] for r in res.results])  # [NCORE, 1, 2]
    ce = float(parts[:, 0, 0].astype(np.float64).sum())
    cnt = float(parts[:, 0, 1].astype(np.float64).sum())
    return np.asarray(ce / max(cnt, 1.0), dtype=np.float32)
